# revision 1
# baseline (speedup 1.0000x reference)
"""TRN2 Bass kernel for nn_AlgebraicBlock (dense transformer block):
MR-norm -> QKV -> ALiBi attention w/ rational softmax -> out-proj residual ->
MR-norm -> rational SwiGLU FFN -> residual.   x: [1, 2048, 1024] f32.

Tensor-parallel over 8 NeuronCores:
  - heads split 2/core for attention (QKV col-split, ALiBi via 4 augmented
    contraction dims, flash-style late softmax normalization via a ones
    column in the PV matmul)
  - AllGather(attn) -> column-split out-projection (exact f32 residual shard)
  - tiny AllReduce for MR-norm column sums (feature dim is sharded)
  - AllGather(h2) -> column-split FFN1 -> row-split FFN2 -> ReduceScatter
All GEMMs bf16 with f32 PSUM accumulation; transposed [feature, T] layout
throughout so all row-softmax/norm reductions happen on the free dim or
through the PE.
"""

import os
import numpy as np
import ml_dtypes

T, C, H, D, F = 2048, 1024, 16, 64, 4096
NCORES = 8
EPS = 1e-6
P = 128
TT = T // 512          # 4 t-tiles of 512
CS = C // P            # 8 c-subtiles
BF = ml_dtypes.bfloat16

TRACE = False          # set True by test.py for neuron-profile timing
LAST_RESULTS = None    # BassKernelResults of the last run (for test.py)

_PROGRAM = None


def _bf16(x):
    return np.asarray(x, dtype=BF)


def _alibi_slopes():
    start = 2.0 ** (-8.0 / H)
    return (start ** np.arange(1, H + 1)).astype(np.float64)


def _prepare_in_maps(x, w_qkv, w_out, w_merged, w3, norm1_w, norm2_w):
    """Host-side sharding + weight preprocessing (layout/precision only)."""
    x = np.asarray(x, np.float32)[0]            # [T, C]
    xT = np.ascontiguousarray(x.T)              # [C, T]
    xt_bf = _bf16(xT)
    slopes = _alibi_slopes()

    pos = np.arange(T, dtype=np.float64)
    t_hi, t_lo = pos // 64, pos % 64

    Wn = np.asarray(w_qkv, np.float32) * np.asarray(norm1_w, np.float32)[None, :]
    w_out = np.asarray(w_out, np.float32)
    wm_n = np.asarray(w_merged, np.float32) * np.asarray(norm2_w, np.float32)[None, :]
    w3 = np.asarray(w3, np.float32)

    in_maps = []
    for i in range(NCORES):
        h0, h1 = 2 * i, 2 * i + 1
        rows = []
        for h in (h0, h1):
            rows.append(Wn[64 * h:64 * h + 64, :] * 0.125)          # q (scaled)
        for h in (h0, h1):
            rows.append(Wn[C + 64 * h:C + 64 * h + 64, :])          # k
        for h in (h0, h1):
            rows.append(Wn[2 * C + 64 * h:2 * C + 64 * h + 64, :])  # v
        wqkv_t = _bf16(np.ascontiguousarray(np.concatenate(rows, 0).T))  # [1024, 384]

        wout_t = _bf16(np.ascontiguousarray(w_out[P * i:P * (i + 1), :].T))  # [1024, 128]

        gsl = slice(512 * i, 512 * (i + 1))
        wm = np.concatenate([wm_n[gsl, :], wm_n[F:][gsl, :]], 0)     # [1024, C]
        wm_t = _bf16(np.ascontiguousarray(wm.T))                     # [1024, 1024]
        w3_t = _bf16(np.ascontiguousarray(w3[:, gsl].T))             # [512, 1024]

        aug_q = np.zeros((2, 4, T), np.float64)
        aug_k = np.zeros((2, 4, T), np.float64)
        for j, h in enumerate((h0, h1)):
            sl = float(_bf16(slopes[h]))
            aug_q[j, 0] = -t_hi
            aug_q[j, 1] = -t_lo
            aug_q[j, 2] = sl * 64
            aug_q[j, 3] = sl
            aug_k[j, 0] = sl * 64
            aug_k[j, 1] = sl
            aug_k[j, 2] = t_hi
            aug_k[j, 3] = t_lo

        maskt = np.zeros((4, P, 512), np.float64)
        for r in range(4):
            s_idx = P * r + np.arange(P)[:, None]
            maskt[r] = (s_idx <= np.arange(512)[None, :]).astype(np.float64)

        in_maps.append({
            "xt_bf": xt_bf,
            "xt_sh": np.ascontiguousarray(xT[P * i:P * (i + 1)]),
            "wqkv_t": wqkv_t,
            "wout_t": wout_t,
            "wm_t": wm_t,
            "w3_t": w3_t,
            "aug_q": _bf16(aug_q),
            "aug_k": _bf16(aug_k),
            "maskt": _bf16(maskt),
        })
    return in_maps


def _build_program():
    import concourse.bass as bass
    import concourse.mybir as mybir
    import concourse.tile as tile
    from concourse import bacc
    from concourse.masks import make_identity

    dt = mybir.dt
    Alu = mybir.AluOpType
    Act = mybir.ActivationFunctionType

    nc = bacc.Bacc("TRN2", target_bir_lowering=False, debug=False,
                   enable_asserts=True, num_devices=NCORES)

    # I/O
    xt_bf = nc.dram_tensor("xt_bf", [C, T], dt.bfloat16, kind="ExternalInput")
    xt_sh = nc.dram_tensor("xt_sh", [P, T], dt.float32, kind="ExternalInput")
    wqkv_t = nc.dram_tensor("wqkv_t", [C, 384], dt.bfloat16, kind="ExternalInput")
    wout_t = nc.dram_tensor("wout_t", [C, P], dt.bfloat16, kind="ExternalInput")
    wm_t = nc.dram_tensor("wm_t", [C, 1024], dt.bfloat16, kind="ExternalInput")
    w3_t = nc.dram_tensor("w3_t", [512, 1024], dt.bfloat16, kind="ExternalInput")
    aug_q = nc.dram_tensor("aug_q", [2, 4, T], dt.bfloat16, kind="ExternalInput")
    aug_k = nc.dram_tensor("aug_k", [2, 4, T], dt.bfloat16, kind="ExternalInput")
    maskt = nc.dram_tensor("maskt", [4, P, 512], dt.bfloat16, kind="ExternalInput")
    out = nc.dram_tensor("out", [P, T], dt.float32, kind="ExternalOutput")

    # internal DRAM (collective bounces)
    attn_in = nc.dram_tensor("attn_in", [TT, P, 512], dt.bfloat16)
    attn_out = nc.dram_tensor("attn_out", [TT, C, 512], dt.bfloat16, addr_space="Shared")
    cs2_in = nc.dram_tensor("cs2_in", [TT, 1, 512], dt.float32)
    cs2_out = nc.dram_tensor("cs2_out", [TT, 1, 512], dt.float32, addr_space="Shared")
    h2_in = nc.dram_tensor("h2_in", [TT, P, 512], dt.bfloat16)
    h2_out = nc.dram_tensor("h2_out", [TT, C, 512], dt.bfloat16, addr_space="Shared")
    rs_in = nc.dram_tensor("rs_in", [2, C, 1024], dt.bfloat16)
    rs_out = nc.dram_tensor("rs_out", [2, P, 1024], dt.bfloat16)

    RG = [list(range(NCORES))]

    with tile.TileContext(nc, num_cores=NCORES) as tc:
        with (
            tc.tile_pool(name="wpool", bufs=1) as wpool,
            tc.tile_pool(name="big", bufs=1) as big,
            tc.tile_pool(name="mid", bufs=1) as mid,
            tc.tile_pool(name="workA", bufs=3) as workA,
            tc.tile_pool(name="workA2", bufs=2) as workA2,
            tc.tile_pool(name="workB", bufs=4) as workB,
            tc.tile_pool(name="workC", bufs=2) as workC,
            tc.tile_pool(name="tiny", bufs=1) as tiny,
            tc.tile_pool(name="tiny2", bufs=1) as tiny2,
            tc.tile_pool(name="psS", bufs=3, space="PSUM") as psS,
            tc.tile_pool(name="psV", bufs=2, space="PSUM") as psV,
        ):
            # ---------- weights / constants (DMA overlaps the AR) ----------
            ident = wpool.tile([P, P], dt.bfloat16)
            make_identity(nc, ident[:])
            wqkv_sb = mid.tile([P, CS, 384], dt.bfloat16, tag="x2", name="wqkv_sb")
            nc.sync.dma_start(wqkv_sb[:], wqkv_t.ap().rearrange("(o p) m -> p o m", p=P))
            xt_sb = big.tile([P, CS, T], dt.bfloat16, tag="big3")
            nc.sync.dma_start(xt_sb[:], xt_bf.ap().rearrange("(o p) t -> p o t", p=P))
            wout_sb = wpool.tile([P, CS, P], dt.bfloat16)
            nc.sync.dma_start(wout_sb[:], wout_t.ap().rearrange("(o p) m -> p o m", p=P))
            wm_sb = wpool.tile([P, CS, 1024], dt.bfloat16)
            nc.sync.dma_start(wm_sb[:], wm_t.ap().rearrange("(o p) m -> p o m", p=P))
            w3_sb = wpool.tile([P, 4, 1024], dt.bfloat16)
            nc.sync.dma_start(w3_sb[:], w3_t.ap().rearrange("(o p) m -> p o m", p=P))
            mask_sb = wpool.tile([P, 4, 512], dt.bfloat16)
            nc.sync.dma_start(mask_sb[:], maskt.ap().rearrange("r p f -> p r f"))

            # ---------- invmean1: local colsums via all-ones matmul ----------
            ones128 = wpool.tile([P, P], dt.bfloat16)
            nc.vector.memset(ones128[:], 1.0)
            ones_sb = wpool.tile([P, 1], dt.bfloat16)
            nc.vector.memset(ones_sb[:], 1.0)
            inv1_sb = mid.tile([P, T], dt.bfloat16, tag="inv1")
            for t2 in range(2):
                tsl = slice(1024 * t2, 1024 * (t2 + 1))
                csp = psS.tile([P, 1024], dt.float32, tag="sc", name="csp1")
                for o in range(CS):
                    axo = workB.tile([P, 1024], dt.bfloat16, tag="u2", name="axo")
                    nc.scalar.activation(axo[:], xt_sb[:, o, tsl], Act.Abs)
                    for half in range(2):
                        hs = slice(512 * half, 512 * (half + 1))
                        nc.tensor.matmul(csp[:, hs], ones128[:], axo[:, hs],
                                         start=(o == 0), stop=(o == CS - 1))
                t1 = workA.tile([P, 1024], dt.float32, tag="a", name="t1")
                nc.vector.tensor_scalar(t1[:], csp[:], 1.0 / C, EPS, Alu.mult, Alu.add)
                rh = workA2.tile([P, 1024], dt.float32, tag="r", name="rh")
                nc.vector.reciprocal_approx_fast(rh[:], t1[:])
                nc.scalar.copy(inv1_sb[:, tsl], rh[:])

            # ---------- QKV GEMM ([128,1024] psum groups) + v transpose ----------
            qa = [mid.tile([P, T], dt.bfloat16, tag=f"qa{j}", name=f"qa{j}")
                  for j in range(2)]
            ka = [mid.tile([P, T], dt.bfloat16, tag=f"ka{j}", name=f"ka{j}")
                  for j in range(2)]
            v_sb = [mid.tile([P, 16, 65], dt.bfloat16, tag=f"v{j}", name=f"v{j}")
                    for j in range(2)]
            for j in range(2):
                nc.vector.memset(qa[j][64:128, :], 0.0)
                nc.vector.memset(ka[j][64:128, :], 0.0)
                nc.sync.dma_start(qa[j][64:68, :], aug_q.ap()[j])
                nc.sync.dma_start(ka[j][64:68, :], aug_k.ap()[j])
                nc.vector.memset(v_sb[j][:, :, 64:65], 1.0)

            for ch in range(3):
                for t4 in range(TT):
                    tsl = slice(512 * t4, 512 * (t4 + 1))
                    pq = psS.tile([P, 512], dt.float32, tag="sc", name="pq")
                    for o in range(CS):
                        nc.tensor.matmul(pq[:], wqkv_sb[:, o, 128 * ch:128 * (ch + 1)],
                                         xt_sb[:, o, tsl],
                                         start=(o == 0), stop=(o == CS - 1))
                    if ch == 0:
                        for j in range(2):
                            nc.vector.scalar_tensor_tensor(
                                qa[j][0:64, tsl], pq[64 * j:64 * j + 64, :], 1.0,
                                inv1_sb[0:64, tsl], Alu.mult, Alu.mult)
                    elif ch == 1:
                        for j in range(2):
                            nc.vector.scalar_tensor_tensor(
                                ka[j][0:64, tsl], pq[64 * j:64 * j + 64, :], 1.0,
                                inv1_sb[0:64, tsl], Alu.mult, Alu.mult)
                    else:
                        vt_w = workC.tile([P, 1024], dt.bfloat16, tag="vtw")
                        nc.vector.scalar_tensor_tensor(
                            vt_w[:, 0:512], pq[:], 1.0, inv1_sb[:, tsl], Alu.mult, Alu.mult)
                        for u in range(4):
                            st = 4 * t4 + u
                            tp = psS.tile([P, P], dt.bfloat16, tag="sc", name="tp")
                            nc.tensor.transpose(tp[:], vt_w[:, P * u:P * (u + 1)], ident[:])
                            for j in range(2):
                                nc.scalar.copy(v_sb[j][:, st, 0:64],
                                               tp[:, 64 * j:64 * j + 64])

            # ---------- attention ----------
            attn_sb = mid.tile([P, T], dt.bfloat16, tag="attn")
            for t4 in range(TT):
                tsl = slice(512 * t4, 512 * (t4 + 1))
                for j in range(2):
                    nst = 4 * t4 + 4
                    npair = nst // 2
                    pv = psV.tile([65, 512], dt.float32, tag="pv", name="pv")
                    sp_t = {}
                    a_t = {}

                    def emit_front(kp, j=j, tsl=tsl):
                        sp = psS.tile([P, 1024], dt.float32, tag="sc", name="sp")
                        for half in range(2):
                            k = 2 * kp + half
                            nc.tensor.matmul(sp[:, 512 * half:512 * (half + 1)],
                                             ka[j][:, P * k:P * (k + 1)],
                                             qa[j][:, tsl], start=True, stop=True)
                        a = workA.tile([P, 1024], dt.float32, tag="a")
                        nc.scalar.activation(a[:], sp[:], Act.Abs)
                        sp_t[kp] = sp
                        a_t[kp] = a

                    LOOKAHEAD = 2
                    for kp in range(min(LOOKAHEAD, npair)):
                        emit_front(kp)
                    for kp in range(npair):
                        if kp + LOOKAHEAD < npair:
                            emit_front(kp + LOOKAHEAD)
                        sp, a = sp_t.pop(kp), a_t.pop(kp)
                        b = workA2.tile([P, 1024], dt.float32, tag="b")
                        if kp % 2 == 0:
                            nc.vector.tensor_scalar(b[:], a[:], 1.0, None, Alu.add)
                        else:
                            nc.scalar.activation(b[:], a[:], Act.Identity, bias=1.0)
                        r = workA2.tile([P, 1024], dt.float32, tag="r", name="r")
                        nc.vector.reciprocal_approx_fast(r[:], b[:])
                        sr = workB.tile([P, 1024], dt.bfloat16, tag="sr")
                        nc.vector.scalar_tensor_tensor(sr[:], sp[:], 1.0, r[:],
                                                       Alu.mult, Alu.mult)
                        u2 = workB.tile([P, 1024], dt.bfloat16, tag="u2")
                        nc.scalar.activation(u2[:], sr[:], Act.Square,
                                             bias=1.0, scale=1.0)
                        p4t = workB.tile([P, 1024], dt.bfloat16, tag="p4")
                        if 2 * kp >= 4 * t4:
                            rr_ = 2 * (kp - 2 * t4)
                            u2m = workB.tile([P, 1024], dt.bfloat16, tag="sr",
                                             name="u2m")
                            nc.vector.tensor_tensor(
                                u2m[:], u2[:], mask_sb[:, rr_:rr_ + 2, :], Alu.mult)
                            nc.vector.tensor_tensor(p4t[:], u2[:], u2m[:], Alu.mult)
                        else:
                            nc.vector.tensor_tensor(p4t[:], u2[:], u2[:], Alu.mult)
                        for half in range(2):
                            k = 2 * kp + half
                            nc.tensor.matmul(pv[:], v_sb[j][:, k, :],
                                             p4t[:, 512 * half:512 * (half + 1)],
                                             start=(k == 0), stop=(k == nst - 1))
                    de = tiny2.tile([1, 512], dt.float32, tag="de")
                    nc.vector.tensor_scalar(de[:], pv[64:65, :], 16.0 * EPS, None, Alu.add)
                    rd = tiny2.tile([1, 512], dt.float32, tag="rd")
                    nc.vector.reciprocal_approx_fast(rd[:], de[:])
                    rdb = tiny2.tile([1, 512], dt.bfloat16, tag="rdb")
                    nc.scalar.copy(rdb[:], rd[:])
                    rdbb = workC.tile([64, 512], dt.bfloat16, tag="rdbb")
                    nc.gpsimd.partition_broadcast(rdbb[:], rdb[:])
                    nc.vector.tensor_tensor(attn_sb[64 * j:64 * j + 64, tsl],
                                            pv[0:64, :], rdbb[:], Alu.mult)
                # chunked AllGather: ship this t-chunk while later chunks compute
                nc.sync.dma_start(attn_in.ap()[t4], attn_sb[:, tsl])
                nc.gpsimd.collective_compute(
                    "AllGather", Alu.bypass, replica_groups=RG,
                    ins=[attn_in.ap()[t4]], outs=[attn_out.ap()[t4]])

            # ---------- out-proj (col-split) + residual, per t-chunk ----------
            af_sb = big.tile([P, CS, T], dt.bfloat16, tag="big3", name="af_sb")
            x2_sb = mid.tile([P, T], dt.float32, tag="x2")
            cs2_sb = tiny.tile([1, T], dt.float32, tag="cs")
            for t4 in range(TT):
                tsl = slice(512 * t4, 512 * (t4 + 1))
                nc.sync.dma_start(af_sb[:, :, tsl],
                                  attn_out.ap()[t4].rearrange("(o p) t -> p o t", p=P))
                yp = psS.tile([P, 512], dt.float32, tag="sc", name="yp")
                for o in range(CS):
                    nc.tensor.matmul(yp[:], wout_sb[:, o, :], af_sb[:, o, tsl],
                                     start=(o == 0), stop=(o == CS - 1))
                xsl = workC.tile([P, 1024], dt.float32, tag="xsl", name="xsl2")
                nc.sync.dma_start(xsl[:, 0:512], xt_sh.ap()[:, tsl])
                nc.vector.tensor_tensor(x2_sb[:, tsl], yp[:], xsl[:, 0:512], Alu.add)
                x2b = workB.tile([P, 512], dt.bfloat16, tag="sr", name="x2b")
                nc.vector.tensor_copy(x2b[:], x2_sb[:, tsl])
                nc.sync.dma_start(h2_in.ap()[t4], x2b[:])
                nc.gpsimd.collective_compute(
                    "AllGather", Alu.bypass, replica_groups=RG,
                    ins=[h2_in.ap()[t4]], outs=[h2_out.ap()[t4]])
                ax2 = workB.tile([P, 512], dt.bfloat16, tag="u2", name="ax2")
                nc.scalar.activation(ax2[:], x2_sb[:, tsl], Act.Abs)
                csp = psS.tile([1, 512], dt.float32, tag="sc", name="csp2")
                nc.tensor.matmul(csp[:], ones_sb[:], ax2[:], start=True, stop=True)
                nc.scalar.copy(cs2_sb[:, tsl], csp[:])
                nc.sync.dma_start(cs2_in.ap()[t4], cs2_sb[0:1, tsl])
                nc.gpsimd.collective_compute(
                    "AllReduce", Alu.add, replica_groups=RG,
                    ins=[cs2_in.ap()[t4]], outs=[cs2_out.ap()[t4]])
            inv2_sb = mid.tile([P, T], dt.bfloat16, tag="inv1", name="inv2_sb")
            s2_sb = tiny.tile([1, T], dt.float32, tag="cs", name="s2_sb")
            r2b_sb = tiny.tile([1, T], dt.bfloat16, tag="r1b", name="r2b_sb")
            for t4 in range(TT):
                tsl = slice(512 * t4, 512 * (t4 + 1))
                nc.sync.dma_start(s2_sb[0:1, tsl], cs2_out.ap()[t4])
                nc.vector.tensor_scalar(s2_sb[0:1, tsl], s2_sb[0:1, tsl],
                                        1.0 / C, EPS, Alu.mult, Alu.add)
                rh2 = workA2.tile([1, 512], dt.float32, tag="r", name="rh2")
                nc.vector.reciprocal_approx_fast(rh2[:], s2_sb[0:1, tsl])
                nc.scalar.copy(r2b_sb[0:1, tsl], rh2[:])
                nc.gpsimd.partition_broadcast(inv2_sb[:, tsl], r2b_sb[0:1, tsl])

            _hid_tags = ["qa0", "qa1", "ka0", "ka1"]
            hid = [mid.tile([P, T], dt.bfloat16, tag=_hid_tags[fc],
                            name=f"hid{fc}") for fc in range(4)]
            for t2 in range(2):
                tsl2 = slice(1024 * t2, 1024 * (t2 + 1))
                x2c = mid.tile([P, CS, 1024], dt.bfloat16, tag="h2c", name="x2c")
                for hf in range(2):
                    t4 = 2 * t2 + hf
                    nc.sync.dma_start(
                        x2c[:, :, 512 * hf:512 * (hf + 1)],
                        h2_out.ap()[t4].rearrange("(o p) t -> p o t", p=P))
                for fc in range(4):
                    for half in range(2):
                        hs = slice(512 * half, 512 * (half + 1))
                        hsl = slice(1024 * t2 + 512 * half,
                                    1024 * t2 + 512 * (half + 1))
                        gp = psS.tile([P, 512], dt.float32, tag="sc", name="gp")
                        vp = psS.tile([P, 512], dt.float32, tag="sc", name="vp")
                        for o in range(CS):
                            nc.tensor.matmul(gp[:],
                                             wm_sb[:, o, 128 * fc:128 * (fc + 1)],
                                             x2c[:, o, hs],
                                             start=(o == 0), stop=(o == CS - 1))
                        for o in range(CS):
                            nc.tensor.matmul(vp[:],
                                             wm_sb[:, o, 512 + 128 * fc:512 + 128 * (fc + 1)],
                                             x2c[:, o, hs],
                                             start=(o == 0), stop=(o == CS - 1))
                        g_sb = workB.tile([P, 512], dt.bfloat16, tag="sr", name="g_sb")
                        nc.vector.scalar_tensor_tensor(g_sb[:], gp[:], 1.0,
                                                       inv2_sb[:, hsl],
                                                       Alu.mult, Alu.mult)
                        v_s = workB.tile([P, 512], dt.bfloat16, tag="u2", name="v_s")
                        nc.vector.scalar_tensor_tensor(v_s[:], vp[:], 1.0,
                                                       inv2_sb[:, hsl],
                                                       Alu.mult, Alu.mult)
                        a2 = workA.tile([P, 512], dt.float32, tag="a", name="a2")
                        nc.scalar.activation(a2[:], g_sb[:], Act.Abs)
                        b2 = workA2.tile([P, 512], dt.float32, tag="b", name="b2")
                        if half == 0:
                            nc.vector.tensor_scalar(b2[:], a2[:], 1.0, None, Alu.add)
                        else:
                            nc.scalar.activation(b2[:], a2[:], Act.Identity, bias=1.0)
                        rr = workA2.tile([P, 512], dt.float32, tag="r", name="rr")
                        nc.vector.reciprocal_approx_fast(rr[:], b2[:])
                        nn = workA2.tile([P, 512], dt.float32, tag="b", name="nn")
                        nc.vector.tensor_scalar(nn[:], g_sb[:], 0.0, 0.5,
                                                Alu.max, Alu.add)
                        nr = workB.tile([P, 512], dt.bfloat16, tag="sr", name="nr")
                        nc.vector.tensor_tensor(nr[:], nn[:], rr[:], Alu.mult)
                        gv = workB.tile([P, 512], dt.bfloat16, tag="p4", name="gv")
                        nc.vector.tensor_tensor(gv[:], g_sb[:], v_s[:], Alu.mult)
                        nc.vector.tensor_tensor(hid[fc][:, hsl], nr[:], gv[:], Alu.mult)

            # ---------- FFN2 (row-split) -> ReduceScatter (2 chunks) ----------
            for half in range(2):
                for jc in range(CS):
                    zs = workC.tile([P, 1024], dt.bfloat16, tag="zs")
                    for h2_ in range(2):
                        hs = slice(512 * h2_, 512 * (h2_ + 1))
                        hsl = slice(1024 * half + 512 * h2_,
                                    1024 * half + 512 * (h2_ + 1))
                        zp = psS.tile([P, 512], dt.float32, tag="sc", name="zp")
                        for o in range(4):
                            nc.tensor.matmul(zp[:],
                                             w3_sb[:, o, 128 * jc:128 * (jc + 1)],
                                             hid[o][:, hsl],
                                             start=(o == 0), stop=(o == 3))
                        nc.scalar.copy(zs[:, hs], zp[:])
                    nc.sync.dma_start(rs_in[half, P * jc:P * (jc + 1), :], zs[:])
                nc.gpsimd.collective_compute(
                    "ReduceScatter", Alu.add, replica_groups=RG,
                    ins=[rs_in.ap()[half]], outs=[rs_out.ap()[half]])

            # ---------- final residual ----------
            rso_sb = mid.tile([P, T], dt.bfloat16, tag="attn", name="rso_sb")
            for half in range(2):
                nc.sync.dma_start(
                    rso_sb[:, 1024 * half:1024 * (half + 1)], rs_out.ap()[half])
            for t2 in range(2):
                tsl = slice(1024 * t2, 1024 * (t2 + 1))
                of = workC.tile([P, 1024], dt.float32, tag="xsl", name="of")
                nc.vector.tensor_tensor(of[:], rso_sb[:, tsl], x2_sb[:, tsl], Alu.add)
                nc.sync.dma_start(out[:, tsl], of[:])

    nc.compile()
    return nc


def _get_program():
    global _PROGRAM
    if _PROGRAM is None:
        _PROGRAM = _build_program()
    return _PROGRAM


def kernel(x, w_qkv, w_out, w_merged, w3, norm1_w, norm2_w):
    global LAST_RESULTS
    from concourse.bass_utils import run_bass_kernel_spmd

    nc = _get_program()
    in_maps = _prepare_in_maps(x, w_qkv, w_out, w_merged, w3, norm1_w, norm2_w)
    res = run_bass_kernel_spmd(nc, in_maps, core_ids=list(range(NCORES)),
                               trace=TRACE)
    LAST_RESULTS = res
    yT = np.concatenate([res.results[i]["out"] for i in range(NCORES)], axis=0)
    return np.ascontiguousarray(yT.T)[None].astype(np.float32)



# revision 17
# speedup vs baseline: 1.1361x; 1.1361x over previous
"""TRN2 Bass kernel for nn_AlgebraicBlock (dense transformer block):
MR-norm -> QKV -> ALiBi attention w/ rational softmax -> out-proj residual ->
MR-norm -> rational SwiGLU FFN -> residual.   x: [1, 2048, 1024] f32.

Tensor-parallel over 8 NeuronCores, fully software-pipelined at 512-column
(t-chunk) granularity:

  it:  0..3   QKV+inv1(it) -> attention(it) -> AllGather(attn it)
  it-1 in 0..3   out-proj(it-1)+residual -> AllGather(x2 it-1)
  it-2 in 0..3   local colsum (NO AllReduce) -> FFN1(it-2) -> FFN2 -> RS(it-2)
  it-3 in 0..3   final residual(it-3) -> DMA out

Rational softmax is collapsed into 2 custom DVE ops per score tile:
  RECIPROCAL_APPROX_FAST(1+|s|)  (scalar engine computes |s|, +1)
  ANT_P4(s, r) = ((s*r + 1)^2)^2       [= 16*rsig(s)^4]
FFN rsig uses ANT_GSR(g, r) = (g*r + 1)*g on the unnormalized gate with
b2 = mean|x2|+eps + |g| so no gate normalization pass is needed; the
inv2^2/2 factor rides the val path (0.5 folded into w3 host-side).
All GEMMs bf16 with f32 PSUM accumulation; transposed [feature, T] layout.
"""

import numpy as np
import ml_dtypes

T, C, H, D, F = 2048, 1024, 16, 64, 4096
NCORES = 8
EPS = 1e-6
P = 128
TT = T // 512          # 4 t-chunks of 512
CS = C // P            # 8 c-subtiles
BF = ml_dtypes.bfloat16

TRACE = False          # set True by test.py for neuron-profile timing
LAST_RESULTS = None    # BassKernelResults of the last run (for test.py)

_PROGRAM = None
_DVE_OPS = None


def _bf16(x):
    return np.asarray(x, dtype=BF)


def _alibi_slopes():
    start = 2.0 ** (-8.0 / H)
    return (start ** np.arange(1, H + 1)).astype(np.float64)


def _register_dve_ops():
    """Register the two fused softmax/ffn DVE ops (idempotent)."""
    global _DVE_OPS
    if _DVE_OPS is not None:
        return _DVE_OPS
    import concourse.dve_ops as dops
    from concourse.dve_spec import Spec, Src0, Src1, One, lower, sq
    from concourse.dve_uop import DveOpSpec

    def _p4ref(in0, in1, c0, c1, c2):
        t = in0.astype(np.float32) * in1.astype(np.float32)
        return (((t + 1.0) ** 2) ** 2).astype(np.float32)

    def _gsrref(in0, in1, c0, c1, c2):
        t = in0.astype(np.float32) * in1.astype(np.float32)
        return ((t + 1.0) * in0.astype(np.float32)).astype(np.float32)

    defs = [
        ("ANT_P4", Spec(body=sq(sq(Src0 * Src1 + One)), reference=_p4ref)),
        ("ANT_GSR", Spec(body=(Src0 * Src1 + One) * Src0, reference=_gsrref)),
    ]
    made = []
    for name, spec in defs:
        if name in dops._SUB_OPCODE_FOR_NAME:
            made.append(next(o for o in dops.OPS if o.name == name))
            continue
        row = max(dops._SUB_OPCODE_FOR_NAME.values()) + 1
        assert row < 0x20
        dops._SUB_OPCODE_FOR_NAME[name] = row
        shas = {}
        for ver in ("v3", "v4"):
            u = lower(spec, ver=ver)
            shas[ver] = DveOpSpec(name=name, opcode=row, uops=u,
                                  rd1_en=True).sha(ver)
        op = dops.DveOp(name, spec, subdim=False, uops_sha=shas)
        dops.OPS.append(op)
        dops.CUSTOM_DVE_SPECS[name] = spec
        made.append(op)
    _DVE_OPS = tuple(made)
    return _DVE_OPS


def _prepare_in_maps(x, w_qkv, w_out, w_merged, w3, norm1_w, norm2_w):
    """Host-side sharding + weight preprocessing (layout/precision only)."""
    x = np.asarray(x, np.float32)[0]            # [T, C]
    xT = np.ascontiguousarray(x.T)              # [C, T]
    xt_bf = _bf16(xT)
    slopes = _alibi_slopes()

    pos = np.arange(T, dtype=np.float64)
    t_hi, t_lo = pos // 64, pos % 64

    Wn = np.asarray(w_qkv, np.float32) * np.asarray(norm1_w, np.float32)[None, :]
    w_out = np.asarray(w_out, np.float32)
    wm_n = np.asarray(w_merged, np.float32) * np.asarray(norm2_w, np.float32)[None, :]
    w3 = np.asarray(w3, np.float32) * 0.5       # rsig half-factor folded here

    in_maps = []
    for i in range(NCORES):
        h0, h1 = 2 * i, 2 * i + 1
        rows = []
        for h in (h0, h1):
            rows.append(Wn[64 * h:64 * h + 64, :] * 0.125)          # q (scaled)
        for h in (h0, h1):
            rows.append(Wn[C + 64 * h:C + 64 * h + 64, :])          # k
        for h in (h0, h1):
            rows.append(Wn[2 * C + 64 * h:2 * C + 64 * h + 64, :])  # v
        wqkv_t = _bf16(np.ascontiguousarray(np.concatenate(rows, 0).T))  # [1024, 384]

        wout_t = _bf16(np.ascontiguousarray(w_out[P * i:P * (i + 1), :].T))  # [1024, 128]

        gsl = slice(512 * i, 512 * (i + 1))
        wm = np.concatenate([wm_n[gsl, :], wm_n[F:][gsl, :]], 0)     # [1024, C]
        wm_t = _bf16(np.ascontiguousarray(wm.T))                     # [1024, 1024]
        w3_t = _bf16(np.ascontiguousarray(w3[:, gsl].T))             # [512, 1024]

        aug_q = np.zeros((2, 4, T), np.float64)
        aug_k = np.zeros((2, 4, T), np.float64)
        for j, h in enumerate((h0, h1)):
            sl = float(_bf16(slopes[h]))
            aug_q[j, 0] = -t_hi
            aug_q[j, 1] = -t_lo
            aug_q[j, 2] = sl * 64
            aug_q[j, 3] = sl
            aug_k[j, 0] = sl * 64
            aug_k[j, 1] = sl
            aug_k[j, 2] = t_hi
            aug_k[j, 3] = t_lo

        maskt = np.zeros((4, P, 512), np.float64)
        for r in range(4):
            s_idx = P * r + np.arange(P)[:, None]
            maskt[r] = (s_idx <= np.arange(512)[None, :]).astype(np.float64)

        in_maps.append({
            "xt_bf": xt_bf,
            "xt_sh": np.ascontiguousarray(xT[P * i:P * (i + 1)]),
            "wqkv_t": wqkv_t,
            "wout_t": wout_t,
            "wm_t": wm_t,
            "w3_t": w3_t,
            "aug_q": _bf16(aug_q),
            "aug_k": _bf16(aug_k),
            "maskt": _bf16(maskt),
        })
    return in_maps


def _build_program():
    import concourse.bass as bass
    import concourse.mybir as mybir
    import concourse.tile as tile
    from concourse import bacc
    from concourse.masks import make_identity

    P4_OP, GSR_OP = _register_dve_ops()

    dt = mybir.dt
    Alu = mybir.AluOpType
    Act = mybir.ActivationFunctionType

    nc = bacc.Bacc("TRN2", target_bir_lowering=False, debug=False,
                   enable_asserts=True, num_devices=NCORES)

    # I/O
    xt_bf = nc.dram_tensor("xt_bf", [C, T], dt.bfloat16, kind="ExternalInput")
    xt_sh = nc.dram_tensor("xt_sh", [P, T], dt.float32, kind="ExternalInput")
    wqkv_t = nc.dram_tensor("wqkv_t", [C, 384], dt.bfloat16, kind="ExternalInput")
    wout_t = nc.dram_tensor("wout_t", [C, P], dt.bfloat16, kind="ExternalInput")
    wm_t = nc.dram_tensor("wm_t", [C, 1024], dt.bfloat16, kind="ExternalInput")
    w3_t = nc.dram_tensor("w3_t", [512, 1024], dt.bfloat16, kind="ExternalInput")
    aug_q = nc.dram_tensor("aug_q", [2, 4, T], dt.bfloat16, kind="ExternalInput")
    aug_k = nc.dram_tensor("aug_k", [2, 4, T], dt.bfloat16, kind="ExternalInput")
    maskt = nc.dram_tensor("maskt", [4, P, 512], dt.bfloat16, kind="ExternalInput")
    out = nc.dram_tensor("out", [P, T], dt.float32, kind="ExternalOutput")

    # internal DRAM (collective bounces)
    warm_in = nc.dram_tensor("warm_in", [1, 64], dt.float32)
    warm_out = nc.dram_tensor("warm_out", [1, 512], dt.float32, addr_space="Shared")
    attn_in = nc.dram_tensor("attn_in", [TT, P, 512], dt.bfloat16)
    attn_out = nc.dram_tensor("attn_out", [TT, C, 512], dt.bfloat16, addr_space="Shared")
    h2_in = nc.dram_tensor("h2_in", [TT, P, 512], dt.bfloat16)
    h2_out = nc.dram_tensor("h2_out", [TT, C, 512], dt.bfloat16, addr_space="Shared")
    rs_in = nc.dram_tensor("rs_in", [TT, C, 512], dt.bfloat16)
    rs_out = nc.dram_tensor("rs_out", [TT, P, 512], dt.bfloat16)

    RG = [list(range(NCORES))]

    with tile.TileContext(nc, num_cores=NCORES) as tc:
        with (
            tc.tile_pool(name="sb", bufs=1) as sb,
            tc.tile_pool(name="wk", bufs=2) as wk,
            tc.tile_pool(name="psS", bufs=2, space="PSUM") as psS,
            tc.tile_pool(name="psV", bufs=2, space="PSUM") as psV,
            tc.tile_pool(name="psF", bufs=2, space="PSUM") as psF,
        ):
            # ---------- constants + weight DMA ----------
            ident = sb.tile([P, P], dt.bfloat16, tag="ident")
            make_identity(nc, ident[:])
            ones128 = sb.tile([P, P], dt.bfloat16, tag="ones")
            nc.vector.memset(ones128[:], 1.0)

            wqkv_sb = sb.tile([P, CS, 384], dt.bfloat16, tag="wqkv")
            nc.sync.dma_start(wqkv_sb[:], wqkv_t.ap().rearrange("(o p) m -> p o m", p=P))

            # warmup collective: absorbs rendezvous/CC cold-start early
            wtiny = sb.tile([1, 64], dt.float32, tag="wtiny")
            nc.vector.memset(wtiny[:], 0.0)
            nc.sync.dma_start(warm_in.ap(), wtiny[:])
            nc.gpsimd.collective_compute(
                "AllGather", Alu.bypass, replica_groups=RG,
                ins=[warm_in.ap()], outs=[warm_out.ap()])

            xt_sb = sb.tile([P, CS, T], dt.bfloat16, tag="big3")
            xt_r = xt_bf.ap().rearrange("(o p) t -> p o t", p=P)
            for c in range(TT):
                tsl = slice(512 * c, 512 * (c + 1))
                nc.sync.dma_start(xt_sb[:, :, tsl], xt_r[:, :, tsl])

            # qa/ka: [64 feat + 4 aug + 60 zero, T]
            qa = [sb.tile([P, T], dt.bfloat16, tag=f"qa{j}", name=f"qa{j}")
                  for j in range(2)]
            ka = [sb.tile([P, T], dt.bfloat16, tag=f"ka{j}", name=f"ka{j}")
                  for j in range(2)]
            v_sb = [sb.tile([P, 16, 65], dt.bfloat16, tag=f"v{j}", name=f"v{j}")
                    for j in range(2)]
            for j in range(2):
                nc.gpsimd.memset(qa[j][64:128, :], 0.0)
                nc.gpsimd.memset(ka[j][64:128, :], 0.0)
                nc.sync.dma_start(qa[j][64:68, :], aug_q.ap()[j])
                nc.sync.dma_start(ka[j][64:68, :], aug_k.ap()[j])
                nc.gpsimd.memset(v_sb[j][:, :, 64:65], 1.0)

            wout_sb = sb.tile([P, CS, P], dt.bfloat16, tag="wout")
            nc.sync.dma_start(wout_sb[:], wout_t.ap().rearrange("(o p) m -> p o m", p=P))
            mask_sb = sb.tile([P, 4, 512], dt.bfloat16, tag="mask")
            nc.sync.dma_start(mask_sb[:], maskt.ap().rearrange("r p f -> p r f"))
            wm_sb = sb.tile([P, CS, 1024], dt.bfloat16, tag="wm")
            nc.sync.dma_start(wm_sb[:], wm_t.ap().rearrange("(o p) m -> p o m", p=P))
            w3_sb = sb.tile([P, 4, 1024], dt.bfloat16, tag="w3")
            nc.sync.dma_start(w3_sb[:], w3_t.ap().rearrange("(o p) m -> p o m", p=P))

            # persistent tiles
            inv1_sb = sb.tile([P, T], dt.bfloat16, tag="inv1")
            x2_sb = sb.tile([P, T], dt.float32, tag="x2f")
            hid = [sb.tile([P, T], dt.bfloat16, tag=f"hid{fc}", name=f"hid{fc}")
                   for fc in range(4)]

            # ---------- pipeline stages ----------

            def qkv_chunk(c):
                tsl = slice(512 * c, 512 * (c + 1))
                # inv1 for this chunk: 1/(mean|x|+eps), broadcast via ones-matmul
                ax = wk.tile([P, CS, 512], dt.bfloat16, tag="ax", bufs=1, name="ax")
                nc.scalar.activation(ax[:], xt_sb[:, :, tsl], Act.Abs)
                csp = psF.tile([P, 512], dt.float32, tag="f", name="csp1")
                for o in range(CS):
                    nc.tensor.matmul(csp[:], ones128[:], ax[:, o, :],
                                     start=(o == 0), stop=(o == CS - 1))
                t1 = wk.tile([P, 512], dt.float32, tag="t1", bufs=1, name="t1")
                nc.vector.tensor_scalar(t1[:], csp[:], 1.0 / C, EPS,
                                        Alu.mult, Alu.add)
                rh = wk.tile([P, 512], dt.float32, tag="rh", bufs=1, name="rh")
                nc.vector.reciprocal_approx_fast(rh[:], t1[:])
                nc.scalar.copy(inv1_sb[:, tsl], rh[:])

                for ch in range(3):
                    pq = psS.tile([P, 512], dt.float32, tag="sc", name="pq")
                    for o in range(CS):
                        nc.tensor.matmul(pq[:], wqkv_sb[:, o, 128 * ch:128 * (ch + 1)],
                                         xt_sb[:, o, tsl],
                                         start=(o == 0), stop=(o == CS - 1))
                    if ch == 0:
                        for j in range(2):
                            nc.vector.scalar_tensor_tensor(
                                qa[j][0:64, tsl], pq[64 * j:64 * j + 64, :], 1.0,
                                inv1_sb[0:64, tsl], Alu.mult, Alu.mult)
                    elif ch == 1:
                        for j in range(2):
                            nc.vector.scalar_tensor_tensor(
                                ka[j][0:64, tsl], pq[64 * j:64 * j + 64, :], 1.0,
                                inv1_sb[0:64, tsl], Alu.mult, Alu.mult)
                    else:
                        vt_w = wk.tile([P, 512], dt.bfloat16, tag="vt", name="vt_w")
                        nc.vector.scalar_tensor_tensor(
                            vt_w[:], pq[:], 1.0, inv1_sb[:, tsl],
                            Alu.mult, Alu.mult)
                        for u in range(4):
                            st = 4 * c + u
                            tp = psF.tile([P, P], dt.bfloat16, tag="f", name="tp")
                            nc.tensor.transpose(tp[:], vt_w[:, P * u:P * (u + 1)],
                                                ident[:])
                            for j in range(2):
                                nc.scalar.copy(v_sb[j][:, st, 0:64],
                                               tp[:, 64 * j:64 * j + 64])

            def attn_chunk(c):
                tsl = slice(512 * c, 512 * (c + 1))
                attn_c = wk.tile([P, 512], dt.bfloat16, tag="attnc", name="attn_c")
                for j in range(2):
                    nst = 4 * c + 4
                    npair = nst // 2
                    pv = psV.tile([65, 512], dt.float32, tag="pv", name="pv")
                    pend = {}

                    def emit(kp, j=j, tsl=tsl):
                        sp = psS.tile([P, 1024], dt.float32, tag="sc", name="sp")
                        for half in range(2):
                            k = 2 * kp + half
                            nc.tensor.matmul(sp[:, 512 * half:512 * (half + 1)],
                                             ka[j][:, P * k:P * (k + 1)],
                                             qa[j][:, tsl], start=True, stop=True)
                        ab = wk.tile([P, 1024], dt.bfloat16, tag="ab", bufs=1,
                                     name="ab")
                        nc.scalar.activation(ab[:], sp[:], Act.Abs)
                        b = wk.tile([P, 1024], dt.float32, tag="b", name="b")
                        nc.scalar.activation(b[:], ab[:], Act.Identity, bias=1.0)
                        pend[kp] = (sp, b)

                    LA = 1
                    for kp in range(min(LA, npair)):
                        emit(kp)
                    for kp in range(npair):
                        if kp + LA < npair:
                            emit(kp + LA)
                        sp, b = pend.pop(kp)
                        r = wk.tile([P, 1024], dt.float32, tag="r", bufs=1,
                                    name="r")
                        nc.vector.reciprocal_approx_fast(r[:], b[:])
                        p4 = wk.tile([P, 1024], dt.bfloat16, tag="p4", name="p4")
                        nc.vector._custom_dve(P4_OP, out=p4[:], in0=sp[:], in1=r[:])
                        if 2 * kp >= 4 * c:
                            rr = 2 * (kp - 2 * c)
                            p4m = wk.tile([P, 1024], dt.bfloat16, tag="p4m",
                                          bufs=1, name="p4m")
                            nc.vector.tensor_tensor(
                                p4m[:], p4[:], mask_sb[:, rr:rr + 2, :], Alu.mult)
                            p4x = p4m
                        else:
                            p4x = p4
                        for half in range(2):
                            k = 2 * kp + half
                            nc.tensor.matmul(pv[:], v_sb[j][:, k, :],
                                             p4x[:, 512 * half:512 * (half + 1)],
                                             start=(k == 0), stop=(k == nst - 1))
                    de = wk.tile([1, 512], dt.float32, tag="de", name="de")
                    nc.vector.tensor_scalar(de[:], pv[64:65, :], 16.0 * EPS, None,
                                            Alu.add)
                    rd = wk.tile([1, 512], dt.float32, tag="rd", name="rd")
                    nc.vector.reciprocal_approx_fast(rd[:], de[:])
                    rdb = wk.tile([1, 512], dt.bfloat16, tag="rdb", name="rdb")
                    nc.scalar.copy(rdb[:], rd[:])
                    rdbb = wk.tile([64, 512], dt.bfloat16, tag="rdbb", name="rdbb")
                    nc.gpsimd.partition_broadcast(rdbb[:], rdb[:])
                    nc.vector.tensor_tensor(attn_c[64 * j:64 * j + 64, :],
                                            pv[0:64, :], rdbb[:], Alu.mult)
                nc.sync.dma_start(attn_in.ap()[c], attn_c[:])
                nc.gpsimd.collective_compute(
                    "AllGather", Alu.bypass, replica_groups=RG,
                    ins=[attn_in.ap()[c]], outs=[attn_out.ap()[c]])

            def oproj_chunk(c):
                tsl = slice(512 * c, 512 * (c + 1))
                af = wk.tile([P, CS, 512], dt.bfloat16, tag="af", bufs=1,
                             name="af")
                nc.sync.dma_start(af[:],
                                  attn_out.ap()[c].rearrange("(o p) t -> p o t", p=P))
                xsl = wk.tile([P, 512], dt.float32, tag="xsl", name="xsl")
                nc.sync.dma_start(xsl[:], xt_sh.ap()[:, tsl])
                yp = psF.tile([P, 512], dt.float32, tag="f", name="yp")
                for o in range(CS):
                    nc.tensor.matmul(yp[:], wout_sb[:, o, :], af[:, o, :],
                                     start=(o == 0), stop=(o == CS - 1))
                nc.vector.tensor_tensor(x2_sb[:, tsl], yp[:], xsl[:], Alu.add)
                x2b = wk.tile([P, 512], dt.bfloat16, tag="x2b", name="x2b")
                nc.scalar.copy(x2b[:], x2_sb[:, tsl])
                nc.sync.dma_start(h2_in.ap()[c], x2b[:])
                nc.gpsimd.collective_compute(
                    "AllGather", Alu.bypass, replica_groups=RG,
                    ins=[h2_in.ap()[c]], outs=[h2_out.ap()[c]])

            def ffn_chunk(c):
                tsl = slice(512 * c, 512 * (c + 1))
                # gathered x2 for this chunk (reuses xt_sb's buffer via slices)
                x2c = xt_sb[:, :, tsl]
                nc.sync.dma_start(x2c,
                                  h2_out.ap()[c].rearrange("(o p) t -> p o t", p=P))
                ax2 = wk.tile([P, CS, 512], dt.bfloat16, tag="ax", bufs=1,
                              name="ax2")
                nc.scalar.activation(ax2[:], x2c, Act.Abs)
                csp2 = psF.tile([P, 512], dt.float32, tag="f", name="csp2")
                for o in range(CS):
                    nc.tensor.matmul(csp2[:], ones128[:], ax2[:, o, :],
                                     start=(o == 0), stop=(o == CS - 1))
                d2 = wk.tile([P, 512], dt.float32, tag="d2", name="d2")
                nc.vector.tensor_scalar(d2[:], csp2[:], 1.0 / C, EPS,
                                        Alu.mult, Alu.add)
                inv2 = wk.tile([P, 512], dt.float32, tag="inv2", bufs=1,
                               name="inv2")
                nc.vector.reciprocal_approx_fast(inv2[:], d2[:])
                isq = wk.tile([P, 512], dt.bfloat16, tag="isq", name="isq")
                nc.vector.tensor_tensor(isq[:], inv2[:], inv2[:], Alu.mult)

                for fc in range(4):
                    gp = psF.tile([P, 512], dt.float32, tag="f", name="gp")
                    for o in range(CS):
                        nc.tensor.matmul(gp[:], wm_sb[:, o, 128 * fc:128 * (fc + 1)],
                                         x2c[:, o, :],
                                         start=(o == 0), stop=(o == CS - 1))
                    vp = psF.tile([P, 512], dt.float32, tag="f", name="vp")
                    for o in range(CS):
                        nc.tensor.matmul(vp[:],
                                         wm_sb[:, o, 512 + 128 * fc:512 + 128 * (fc + 1)],
                                         x2c[:, o, :],
                                         start=(o == 0), stop=(o == CS - 1))
                    ag = wk.tile([P, 512], dt.bfloat16, tag="ag", bufs=1, name="ag")
                    nc.scalar.activation(ag[:], gp[:], Act.Abs)
                    b2 = wk.tile([P, 512], dt.float32, tag="b2", name="b2")
                    nc.gpsimd.tensor_tensor(b2[:], ag[:], d2[:], Alu.add)
                    r2 = wk.tile([P, 512], dt.float32, tag="r2", bufs=1,
                                 name="r2")
                    nc.vector.reciprocal_approx_fast(r2[:], b2[:])
                    gs = wk.tile([P, 512], dt.bfloat16, tag="gs", bufs=1,
                                 name="gs")
                    nc.vector._custom_dve(GSR_OP, out=gs[:], in0=gp[:], in1=r2[:])
                    vb = wk.tile([P, 512], dt.bfloat16, tag="vb", bufs=1, name="vb")
                    nc.scalar.copy(vb[:], vp[:])
                    vbs = wk.tile([P, 512], dt.bfloat16, tag="vbs", name="vbs")
                    nc.gpsimd.tensor_tensor(vbs[:], vb[:], isq[:], Alu.mult)
                    nc.vector.tensor_tensor(hid[fc][:, tsl], gs[:], vbs[:],
                                            Alu.mult)

                # FFN2 (row-split partials) for this chunk -> ReduceScatter
                for jc in range(CS):
                    zp = psF.tile([P, 512], dt.float32, tag="f", name="zp")
                    for o in range(4):
                        nc.tensor.matmul(zp[:], w3_sb[:, o, 128 * jc:128 * (jc + 1)],
                                         hid[o][:, tsl],
                                         start=(o == 0), stop=(o == 3))
                    zs = wk.tile([P, 512], dt.bfloat16, tag="zs", name="zs")
                    if jc % 2 == 0:
                        nc.scalar.copy(zs[:], zp[:])
                    else:
                        nc.vector.tensor_copy(zs[:], zp[:])
                    nc.sync.dma_start(rs_in[c, P * jc:P * (jc + 1), :], zs[:])
                nc.gpsimd.collective_compute(
                    "ReduceScatter", Alu.add, replica_groups=RG,
                    ins=[rs_in.ap()[c]], outs=[rs_out.ap()[c]])

            def resid_chunk(c):
                tsl = slice(512 * c, 512 * (c + 1))
                rso = wk.tile([P, 512], dt.bfloat16, tag="rso", name="rso")
                nc.sync.dma_start(rso[:], rs_out.ap()[c])
                of = wk.tile([P, 512], dt.float32, tag="of", name="of")
                nc.vector.tensor_tensor(of[:], rso[:], x2_sb[:, tsl], Alu.add)
                nc.sync.dma_start(out[:, tsl], of[:])

            for it in range(TT + 3):
                with nc.named_scope(f"it{it}"):
                    if it < TT:
                        qkv_chunk(it)
                        attn_chunk(it)
                    if 1 <= it <= TT:
                        oproj_chunk(it - 1)
                    if 2 <= it <= TT + 1:
                        ffn_chunk(it - 2)
                    if 3 <= it <= TT + 2:
                        resid_chunk(it - 3)

    nc.compile()
    return nc


def _get_program():
    global _PROGRAM
    if _PROGRAM is None:
        _PROGRAM = _build_program()
    return _PROGRAM


def kernel(x, w_qkv, w_out, w_merged, w3, norm1_w, norm2_w):
    global LAST_RESULTS
    from concourse.bass_utils import run_bass_kernel_spmd

    nc = _get_program()
    in_maps = _prepare_in_maps(x, w_qkv, w_out, w_merged, w3, norm1_w, norm2_w)
    res = run_bass_kernel_spmd(nc, in_maps, core_ids=list(range(NCORES)),
                               trace=TRACE)
    LAST_RESULTS = res
    yT = np.concatenate([res.results[i]["out"] for i in range(NCORES)], axis=0)
    return np.ascontiguousarray(yT.T)[None].astype(np.float32)


# revision 18
# speedup vs baseline: 1.1478x; 1.0103x over previous
"""TRN2 Bass kernel for nn_AlgebraicBlock (dense transformer block):
MR-norm -> QKV -> ALiBi attention w/ rational softmax -> out-proj residual ->
MR-norm -> rational SwiGLU FFN -> residual.   x: [1, 2048, 1024] f32.

Tensor-parallel over 8 NeuronCores, fully software-pipelined at 512-column
(t-chunk) granularity:

  it:  0..3   QKV+inv1(it) -> attention(it) -> AllGather(attn it)
  it-1 in 0..3   out-proj(it-1)+residual -> AllGather(x2 it-1)
  it-2 in 0..3   local colsum (NO AllReduce) -> FFN1(it-2) -> FFN2 -> RS(it-2)
  it-3 in 0..3   final residual(it-3) -> DMA out

Rational softmax is collapsed into 2 custom DVE ops per score tile:
  RECIPROCAL_APPROX_FAST(1+|s|)  (scalar engine computes |s|, +1)
  ANT_P4(s, r) = ((s*r + 1)^2)^2       [= 16*rsig(s)^4]
FFN rsig uses ANT_GSR(g, r) = (g*r + 1)*g on the unnormalized gate with
b2 = mean|x2|+eps + |g| so no gate normalization pass is needed; the
inv2^2/2 factor rides the val path (0.5 folded into w3 host-side).
All GEMMs bf16 with f32 PSUM accumulation; transposed [feature, T] layout.
"""

import numpy as np
import ml_dtypes

T, C, H, D, F = 2048, 1024, 16, 64, 4096
NCORES = 8
EPS = 1e-6
P = 128
TT = T // 512          # 4 t-chunks of 512
CS = C // P            # 8 c-subtiles
BF = ml_dtypes.bfloat16

TRACE = False          # set True by test.py for neuron-profile timing
LAST_RESULTS = None    # BassKernelResults of the last run (for test.py)

_PROGRAM = None
_DVE_OPS = None


def _bf16(x):
    return np.asarray(x, dtype=BF)


def _alibi_slopes():
    start = 2.0 ** (-8.0 / H)
    return (start ** np.arange(1, H + 1)).astype(np.float64)


def _register_dve_ops():
    """Register the two fused softmax/ffn DVE ops (idempotent)."""
    global _DVE_OPS
    if _DVE_OPS is not None:
        return _DVE_OPS
    import concourse.dve_ops as dops
    from concourse.dve_spec import Spec, Src0, Src1, One, lower, sq
    from concourse.dve_uop import DveOpSpec

    def _p4ref(in0, in1, c0, c1, c2):
        t = in0.astype(np.float32) * in1.astype(np.float32)
        return (((t + 1.0) ** 2) ** 2).astype(np.float32)

    def _gsrref(in0, in1, c0, c1, c2):
        t = in0.astype(np.float32) * in1.astype(np.float32)
        return ((t + 1.0) * in0.astype(np.float32)).astype(np.float32)

    defs = [
        ("ANT_P4", Spec(body=sq(sq(Src0 * Src1 + One)), reference=_p4ref)),
        ("ANT_GSR", Spec(body=(Src0 * Src1 + One) * Src0, reference=_gsrref)),
    ]
    made = []
    for name, spec in defs:
        if name in dops._SUB_OPCODE_FOR_NAME:
            made.append(next(o for o in dops.OPS if o.name == name))
            continue
        row = max(dops._SUB_OPCODE_FOR_NAME.values()) + 1
        assert row < 0x20
        dops._SUB_OPCODE_FOR_NAME[name] = row
        shas = {}
        for ver in ("v3", "v4"):
            u = lower(spec, ver=ver)
            shas[ver] = DveOpSpec(name=name, opcode=row, uops=u,
                                  rd1_en=True).sha(ver)
        op = dops.DveOp(name, spec, subdim=False, uops_sha=shas)
        dops.OPS.append(op)
        dops.CUSTOM_DVE_SPECS[name] = spec
        made.append(op)
    _DVE_OPS = tuple(made)
    return _DVE_OPS


def _prepare_in_maps(x, w_qkv, w_out, w_merged, w3, norm1_w, norm2_w):
    """Host-side sharding + weight preprocessing (layout/precision only)."""
    x = np.asarray(x, np.float32)[0]            # [T, C]
    xT = np.ascontiguousarray(x.T)              # [C, T]
    xt_bf = _bf16(xT)
    slopes = _alibi_slopes()

    pos = np.arange(T, dtype=np.float64)
    t_hi, t_lo = pos // 64, pos % 64

    Wn = np.asarray(w_qkv, np.float32) * np.asarray(norm1_w, np.float32)[None, :]
    w_out = np.asarray(w_out, np.float32)
    wm_n = np.asarray(w_merged, np.float32) * np.asarray(norm2_w, np.float32)[None, :]
    w3 = np.asarray(w3, np.float32) * 0.5       # rsig half-factor folded here

    in_maps = []
    for i in range(NCORES):
        h0, h1 = 2 * i, 2 * i + 1
        rows = []
        for h in (h0, h1):
            rows.append(Wn[64 * h:64 * h + 64, :] * 0.125)          # q (scaled)
        for h in (h0, h1):
            rows.append(Wn[C + 64 * h:C + 64 * h + 64, :])          # k
        for h in (h0, h1):
            rows.append(Wn[2 * C + 64 * h:2 * C + 64 * h + 64, :])  # v
        wqkv_t = _bf16(np.ascontiguousarray(np.concatenate(rows, 0).T))  # [1024, 384]

        wout_t = _bf16(np.ascontiguousarray(w_out[P * i:P * (i + 1), :].T))  # [1024, 128]

        gsl = slice(512 * i, 512 * (i + 1))
        wm = np.concatenate([wm_n[gsl, :], wm_n[F:][gsl, :]], 0)     # [1024, C]
        wm_t = _bf16(np.ascontiguousarray(wm.T))                     # [1024, 1024]
        w3_t = _bf16(np.ascontiguousarray(w3[:, gsl].T))             # [512, 1024]

        aug_q = np.zeros((2, 4, T), np.float64)
        aug_k = np.zeros((2, 4, T), np.float64)
        for j, h in enumerate((h0, h1)):
            sl = float(_bf16(slopes[h]))
            aug_q[j, 0] = -t_hi
            aug_q[j, 1] = -t_lo
            aug_q[j, 2] = sl * 64
            aug_q[j, 3] = sl
            aug_k[j, 0] = sl * 64
            aug_k[j, 1] = sl
            aug_k[j, 2] = t_hi
            aug_k[j, 3] = t_lo

        maskt = np.zeros((4, P, 512), np.float64)
        for r in range(4):
            s_idx = P * r + np.arange(P)[:, None]
            maskt[r] = (s_idx <= np.arange(512)[None, :]).astype(np.float64)

        in_maps.append({
            "xt_bf": xt_bf,
            "xt_sh": np.ascontiguousarray(xT[P * i:P * (i + 1)]),
            "wqkv_t": wqkv_t,
            "wout_t": wout_t,
            "wm_t": wm_t,
            "w3_t": w3_t,
            "aug_q": _bf16(aug_q),
            "aug_k": _bf16(aug_k),
            "maskt": _bf16(maskt),
        })
    return in_maps


def _build_program():
    import concourse.bass as bass
    import concourse.mybir as mybir
    import concourse.tile as tile
    from concourse import bacc
    from concourse.masks import make_identity

    P4_OP, GSR_OP = _register_dve_ops()

    dt = mybir.dt
    Alu = mybir.AluOpType
    Act = mybir.ActivationFunctionType

    nc = bacc.Bacc("TRN2", target_bir_lowering=False, debug=False,
                   enable_asserts=True, num_devices=NCORES)

    # I/O
    xt_bf = nc.dram_tensor("xt_bf", [C, T], dt.bfloat16, kind="ExternalInput")
    xt_sh = nc.dram_tensor("xt_sh", [P, T], dt.float32, kind="ExternalInput")
    wqkv_t = nc.dram_tensor("wqkv_t", [C, 384], dt.bfloat16, kind="ExternalInput")
    wout_t = nc.dram_tensor("wout_t", [C, P], dt.bfloat16, kind="ExternalInput")
    wm_t = nc.dram_tensor("wm_t", [C, 1024], dt.bfloat16, kind="ExternalInput")
    w3_t = nc.dram_tensor("w3_t", [512, 1024], dt.bfloat16, kind="ExternalInput")
    aug_q = nc.dram_tensor("aug_q", [2, 4, T], dt.bfloat16, kind="ExternalInput")
    aug_k = nc.dram_tensor("aug_k", [2, 4, T], dt.bfloat16, kind="ExternalInput")
    maskt = nc.dram_tensor("maskt", [4, P, 512], dt.bfloat16, kind="ExternalInput")
    out = nc.dram_tensor("out", [P, T], dt.float32, kind="ExternalOutput")

    # internal DRAM (collective bounces)
    warm_in = nc.dram_tensor("warm_in", [1, 64], dt.float32)
    warm_out = nc.dram_tensor("warm_out", [1, 512], dt.float32, addr_space="Shared")
    attn_in = nc.dram_tensor("attn_in", [TT, P, 512], dt.bfloat16)
    attn_out = nc.dram_tensor("attn_out", [TT, C, 512], dt.bfloat16, addr_space="Shared")
    h2_in = nc.dram_tensor("h2_in", [TT, P, 512], dt.bfloat16)
    h2_out = nc.dram_tensor("h2_out", [TT, C, 512], dt.bfloat16, addr_space="Shared")
    rs_in = nc.dram_tensor("rs_in", [TT, C, 512], dt.bfloat16)
    rs_out = nc.dram_tensor("rs_out", [TT, P, 512], dt.bfloat16)

    RG = [list(range(NCORES))]

    with tile.TileContext(nc, num_cores=NCORES) as tc:
        with (
            tc.tile_pool(name="sb", bufs=1) as sb,
            tc.tile_pool(name="wk", bufs=2) as wk,
            tc.tile_pool(name="psS", bufs=2, space="PSUM") as psS,
            tc.tile_pool(name="psV", bufs=2, space="PSUM") as psV,
            tc.tile_pool(name="psF", bufs=2, space="PSUM") as psF,
        ):
            # ---------- constants + weight DMA ----------
            ident = sb.tile([P, P], dt.bfloat16, tag="ident")
            make_identity(nc, ident[:])
            ones128 = sb.tile([P, P], dt.bfloat16, tag="ones")
            nc.vector.memset(ones128[:], 1.0)

            wqkv_sb = sb.tile([P, CS, 384], dt.bfloat16, tag="wqkv")
            nc.sync.dma_start(wqkv_sb[:], wqkv_t.ap().rearrange("(o p) m -> p o m", p=P))

            # warmup collective: absorbs rendezvous/CC cold-start early
            wtiny = sb.tile([1, 64], dt.float32, tag="wtiny")
            nc.vector.memset(wtiny[:], 0.0)
            nc.sync.dma_start(warm_in.ap(), wtiny[:])
            nc.gpsimd.collective_compute(
                "AllGather", Alu.bypass, replica_groups=RG,
                ins=[warm_in.ap()], outs=[warm_out.ap()])

            xt_sb = sb.tile([P, CS, T], dt.bfloat16, tag="big3")
            xt_r = xt_bf.ap().rearrange("(o p) t -> p o t", p=P)
            for c in range(TT):
                tsl = slice(512 * c, 512 * (c + 1))
                nc.sync.dma_start(xt_sb[:, :, tsl], xt_r[:, :, tsl])

            # qa/ka: [64 feat + 4 aug + 60 zero, T]
            qa = [sb.tile([P, T], dt.bfloat16, tag=f"qa{j}", name=f"qa{j}")
                  for j in range(2)]
            ka = [sb.tile([P, T], dt.bfloat16, tag=f"ka{j}", name=f"ka{j}")
                  for j in range(2)]
            v_sb = [sb.tile([P, 16, 65], dt.bfloat16, tag=f"v{j}", name=f"v{j}")
                    for j in range(2)]
            for j in range(2):
                nc.gpsimd.memset(qa[j][64:128, :], 0.0)
                nc.gpsimd.memset(ka[j][64:128, :], 0.0)
                nc.sync.dma_start(qa[j][64:68, :], aug_q.ap()[j])
                nc.sync.dma_start(ka[j][64:68, :], aug_k.ap()[j])
                nc.gpsimd.memset(v_sb[j][:, :, 64:65], 1.0)

            wout_sb = sb.tile([P, CS, P], dt.bfloat16, tag="wout")
            nc.sync.dma_start(wout_sb[:], wout_t.ap().rearrange("(o p) m -> p o m", p=P))
            mask_sb = sb.tile([P, 4, 512], dt.bfloat16, tag="mask")
            nc.sync.dma_start(mask_sb[:], maskt.ap().rearrange("r p f -> p r f"))
            wm_sb = sb.tile([P, CS, 1024], dt.bfloat16, tag="wm")
            nc.sync.dma_start(wm_sb[:], wm_t.ap().rearrange("(o p) m -> p o m", p=P))
            w3_sb = sb.tile([P, 4, 1024], dt.bfloat16, tag="w3")
            nc.sync.dma_start(w3_sb[:], w3_t.ap().rearrange("(o p) m -> p o m", p=P))

            # persistent tiles
            inv1_sb = sb.tile([P, T], dt.bfloat16, tag="inv1")
            x2_sb = sb.tile([P, T], dt.float32, tag="x2f")
            hid = [sb.tile([P, T], dt.bfloat16, tag=f"hid{fc}", name=f"hid{fc}")
                   for fc in range(4)]

            # ---------- pipeline stages ----------

            def qkv_chunk(c):
                tsl = slice(512 * c, 512 * (c + 1))
                # inv1 for this chunk: 1/(mean|x|+eps), broadcast via ones-matmul
                ax = wk.tile([P, CS, 512], dt.bfloat16, tag="ax", bufs=1, name="ax")
                nc.scalar.activation(ax[:], xt_sb[:, :, tsl], Act.Abs)
                csp = psF.tile([P, 512], dt.float32, tag="f", name="csp1")
                for o in range(CS):
                    nc.tensor.matmul(csp[:], ones128[:], ax[:, o, :],
                                     start=(o == 0), stop=(o == CS - 1))
                t1 = wk.tile([P, 512], dt.float32, tag="t1", bufs=1, name="t1")
                nc.vector.tensor_scalar(t1[:], csp[:], 1.0 / C, EPS,
                                        Alu.mult, Alu.add)
                rh = wk.tile([P, 512], dt.float32, tag="rh", bufs=1, name="rh")
                nc.vector.reciprocal_approx_fast(rh[:], t1[:])
                nc.scalar.copy(inv1_sb[:, tsl], rh[:])

                for ch in range(3):
                    pq = psS.tile([P, 512], dt.float32, tag="sc", name="pq")
                    for o in range(CS):
                        nc.tensor.matmul(pq[:], wqkv_sb[:, o, 128 * ch:128 * (ch + 1)],
                                         xt_sb[:, o, tsl],
                                         start=(o == 0), stop=(o == CS - 1))
                    if ch == 0:
                        for j in range(2):
                            nc.vector.scalar_tensor_tensor(
                                qa[j][0:64, tsl], pq[64 * j:64 * j + 64, :], 1.0,
                                inv1_sb[0:64, tsl], Alu.mult, Alu.mult)
                    elif ch == 1:
                        for j in range(2):
                            nc.vector.scalar_tensor_tensor(
                                ka[j][0:64, tsl], pq[64 * j:64 * j + 64, :], 1.0,
                                inv1_sb[0:64, tsl], Alu.mult, Alu.mult)
                    else:
                        vt_w = wk.tile([P, 512], dt.bfloat16, tag="vt", name="vt_w")
                        nc.vector.scalar_tensor_tensor(
                            vt_w[:], pq[:], 1.0, inv1_sb[:, tsl],
                            Alu.mult, Alu.mult)
                        for u in range(4):
                            st = 4 * c + u
                            tp = psF.tile([P, P], dt.bfloat16, tag="f", name="tp")
                            nc.tensor.transpose(tp[:], vt_w[:, P * u:P * (u + 1)],
                                                ident[:])
                            for j in range(2):
                                nc.scalar.copy(v_sb[j][:, st, 0:64],
                                               tp[:, 64 * j:64 * j + 64])

            def attn_chunk(c):
                tsl = slice(512 * c, 512 * (c + 1))
                attn_c = wk.tile([P, 512], dt.bfloat16, tag="attnc", name="attn_c")
                for j in range(2):
                    nst = 4 * c + 4
                    npair = nst // 2
                    pv = psV.tile([65, 512], dt.float32, tag="pv", name="pv")
                    pend = {}

                    def emit(kp, j=j, tsl=tsl):
                        sp = psS.tile([P, 1024], dt.float32, tag="sc", name="sp")
                        for half in range(2):
                            k = 2 * kp + half
                            nc.tensor.matmul(sp[:, 512 * half:512 * (half + 1)],
                                             ka[j][:, P * k:P * (k + 1)],
                                             qa[j][:, tsl], start=True, stop=True)
                        ab = wk.tile([P, 1024], dt.bfloat16, tag="ab", bufs=1,
                                     name="ab")
                        nc.scalar.activation(ab[:], sp[:], Act.Abs)
                        b = wk.tile([P, 1024], dt.float32, tag="b", name="b")
                        nc.scalar.activation(b[:], ab[:], Act.Identity, bias=1.0)
                        pend[kp] = (sp, b)

                    LA = 1
                    for kp in range(min(LA, npair)):
                        emit(kp)
                    for kp in range(npair):
                        if kp + LA < npair:
                            emit(kp + LA)
                        sp, b = pend.pop(kp)
                        r = wk.tile([P, 1024], dt.float32, tag="r", bufs=1,
                                    name="r")
                        nc.vector.reciprocal_approx_fast(r[:], b[:])
                        p4 = wk.tile([P, 1024], dt.bfloat16, tag="p4", name="p4")
                        nc.vector._custom_dve(P4_OP, out=p4[:], in0=sp[:], in1=r[:])
                        if 2 * kp >= 4 * c:
                            rr = 2 * (kp - 2 * c)
                            p4m = wk.tile([P, 1024], dt.bfloat16, tag="p4m",
                                          bufs=1, name="p4m")
                            nc.vector.tensor_tensor(
                                p4m[:], p4[:], mask_sb[:, rr:rr + 2, :], Alu.mult)
                            p4x = p4m
                        else:
                            p4x = p4
                        for half in range(2):
                            k = 2 * kp + half
                            nc.tensor.matmul(pv[:], v_sb[j][:, k, :],
                                             p4x[:, 512 * half:512 * (half + 1)],
                                             start=(k == 0), stop=(k == nst - 1))
                    de = wk.tile([1, 512], dt.float32, tag="de", name="de")
                    nc.vector.tensor_scalar(de[:], pv[64:65, :], 16.0 * EPS, None,
                                            Alu.add)
                    rd = wk.tile([1, 512], dt.float32, tag="rd", name="rd")
                    nc.vector.reciprocal_approx_fast(rd[:], de[:])
                    rdb = wk.tile([1, 512], dt.bfloat16, tag="rdb", name="rdb")
                    nc.scalar.copy(rdb[:], rd[:])
                    rdbb = wk.tile([64, 512], dt.bfloat16, tag="rdbb", name="rdbb")
                    nc.gpsimd.partition_broadcast(rdbb[:], rdb[:])
                    nc.vector.tensor_tensor(attn_c[64 * j:64 * j + 64, :],
                                            pv[0:64, :], rdbb[:], Alu.mult)
                nc.sync.dma_start(attn_in.ap()[c], attn_c[:])
                nc.gpsimd.collective_compute(
                    "AllGather", Alu.bypass, replica_groups=RG,
                    ins=[attn_in.ap()[c]], outs=[attn_out.ap()[c]])

            def oproj_chunk(c):
                tsl = slice(512 * c, 512 * (c + 1))
                af = wk.tile([P, CS, 512], dt.bfloat16, tag="af", bufs=1,
                             name="af")
                nc.sync.dma_start(af[:],
                                  attn_out.ap()[c].rearrange("(o p) t -> p o t", p=P))
                xsl = wk.tile([P, 512], dt.float32, tag="xsl", name="xsl")
                nc.sync.dma_start(xsl[:], xt_sh.ap()[:, tsl])
                yp = psF.tile([P, 512], dt.float32, tag="f", name="yp")
                for o in range(CS):
                    nc.tensor.matmul(yp[:], wout_sb[:, o, :], af[:, o, :],
                                     start=(o == 0), stop=(o == CS - 1))
                nc.vector.tensor_tensor(x2_sb[:, tsl], yp[:], xsl[:], Alu.add)
                x2b = wk.tile([P, 512], dt.bfloat16, tag="x2b", name="x2b")
                nc.scalar.copy(x2b[:], x2_sb[:, tsl])
                nc.sync.dma_start(h2_in.ap()[c], x2b[:])
                nc.gpsimd.collective_compute(
                    "AllGather", Alu.bypass, replica_groups=RG,
                    ins=[h2_in.ap()[c]], outs=[h2_out.ap()[c]])

            def ffn_chunk(c):
                tsl = slice(512 * c, 512 * (c + 1))
                # gathered x2 for this chunk (reuses xt_sb's buffer via slices)
                x2c = xt_sb[:, :, tsl]
                nc.sync.dma_start(x2c,
                                  h2_out.ap()[c].rearrange("(o p) t -> p o t", p=P))
                ax2 = wk.tile([P, CS, 512], dt.bfloat16, tag="ax", bufs=1,
                              name="ax2")
                nc.scalar.activation(ax2[:], x2c, Act.Abs)
                csp2 = psF.tile([P, 512], dt.float32, tag="f", name="csp2")
                for o in range(CS):
                    nc.tensor.matmul(csp2[:], ones128[:], ax2[:, o, :],
                                     start=(o == 0), stop=(o == CS - 1))
                d2 = wk.tile([P, 512], dt.float32, tag="d2", name="d2")
                nc.vector.tensor_scalar(d2[:], csp2[:], 1.0 / C, EPS,
                                        Alu.mult, Alu.add)
                inv2 = wk.tile([P, 512], dt.float32, tag="inv2", bufs=1,
                               name="inv2")
                nc.vector.reciprocal_approx_fast(inv2[:], d2[:])
                isq = wk.tile([P, 512], dt.bfloat16, tag="isq", name="isq")
                nc.vector.tensor_tensor(isq[:], inv2[:], inv2[:], Alu.mult)

                for fc in range(4):
                    gp = psF.tile([P, 512], dt.float32, tag="f", name="gp")
                    for o in range(CS):
                        nc.tensor.matmul(gp[:], wm_sb[:, o, 128 * fc:128 * (fc + 1)],
                                         x2c[:, o, :],
                                         start=(o == 0), stop=(o == CS - 1))
                    vp = psF.tile([P, 512], dt.float32, tag="f", name="vp")
                    for o in range(CS):
                        nc.tensor.matmul(vp[:],
                                         wm_sb[:, o, 512 + 128 * fc:512 + 128 * (fc + 1)],
                                         x2c[:, o, :],
                                         start=(o == 0), stop=(o == CS - 1))
                    ag = wk.tile([P, 512], dt.bfloat16, tag="ag", bufs=1, name="ag")
                    nc.scalar.activation(ag[:], gp[:], Act.Abs)
                    b2 = wk.tile([P, 512], dt.float32, tag="b2", name="b2")
                    nc.gpsimd.tensor_tensor(b2[:], ag[:], d2[:], Alu.add)
                    r2 = wk.tile([P, 512], dt.float32, tag="r2", bufs=1,
                                 name="r2")
                    nc.vector.reciprocal_approx_fast(r2[:], b2[:])
                    gs = wk.tile([P, 512], dt.bfloat16, tag="gs", bufs=1,
                                 name="gs")
                    nc.vector._custom_dve(GSR_OP, out=gs[:], in0=gp[:], in1=r2[:])
                    vb = wk.tile([P, 512], dt.bfloat16, tag="vb", bufs=1, name="vb")
                    nc.scalar.copy(vb[:], vp[:])
                    vbs = wk.tile([P, 512], dt.bfloat16, tag="vbs", name="vbs")
                    nc.gpsimd.tensor_tensor(vbs[:], vb[:], isq[:], Alu.mult)
                    nc.vector.tensor_tensor(hid[fc][:, tsl], gs[:], vbs[:],
                                            Alu.mult)

                # FFN2 (row-split partials) for this chunk -> ReduceScatter
                for jc in range(CS):
                    zp = psF.tile([P, 512], dt.float32, tag="f", name="zp")
                    for o in range(4):
                        nc.tensor.matmul(zp[:], w3_sb[:, o, 128 * jc:128 * (jc + 1)],
                                         hid[o][:, tsl],
                                         start=(o == 0), stop=(o == 3))
                    zs = wk.tile([P, 512], dt.bfloat16, tag="zs", name="zs")
                    if jc % 2 == 0:
                        nc.scalar.copy(zs[:], zp[:])
                    else:
                        nc.vector.tensor_copy(zs[:], zp[:])
                    nc.sync.dma_start(rs_in[c, P * jc:P * (jc + 1), :], zs[:])
                nc.gpsimd.collective_compute(
                    "ReduceScatter", Alu.add, replica_groups=RG,
                    ins=[rs_in.ap()[c]], outs=[rs_out.ap()[c]])

            def resid_chunk(c):
                tsl = slice(512 * c, 512 * (c + 1))
                rso = wk.tile([P, 512], dt.bfloat16, tag="rso", name="rso")
                nc.sync.dma_start(rso[:], rs_out.ap()[c])
                of = wk.tile([P, 512], dt.float32, tag="of", name="of")
                nc.vector.tensor_tensor(of[:], rso[:], x2_sb[:, tsl], Alu.add)
                nc.sync.dma_start(out[:, tsl], of[:])

            for it in range(TT + 3):
                with nc.named_scope(f"it{it}"):
                    if it < TT:
                        qkv_chunk(it)
                    if 1 <= it <= TT:
                        oproj_chunk(it - 1)
                    if it < TT:
                        attn_chunk(it)
                    if 2 <= it <= TT + 1:
                        ffn_chunk(it - 2)
                    if 3 <= it <= TT + 2:
                        resid_chunk(it - 3)

    nc.compile()
    return nc


def _get_program():
    global _PROGRAM
    if _PROGRAM is None:
        _PROGRAM = _build_program()
    return _PROGRAM


def kernel(x, w_qkv, w_out, w_merged, w3, norm1_w, norm2_w):
    global LAST_RESULTS
    from concourse.bass_utils import run_bass_kernel_spmd

    nc = _get_program()
    in_maps = _prepare_in_maps(x, w_qkv, w_out, w_merged, w3, norm1_w, norm2_w)
    res = run_bass_kernel_spmd(nc, in_maps, core_ids=list(range(NCORES)),
                               trace=TRACE)
    LAST_RESULTS = res
    yT = np.concatenate([res.results[i]["out"] for i in range(NCORES)], axis=0)
    return np.ascontiguousarray(yT.T)[None].astype(np.float32)
